# revision 44
# baseline (speedup 1.0000x reference)
"""2-layer GraphSAGE (mean aggr) on 8 Trainium2 NeuronCores.

Strategy: partition destination nodes across cores (graph parallel).
Segment-mean is computed as TensorE matmuls: for each 128-slot column of
gathered source rows M [128e, D], a routing matrix S [128e, W] (one-hot
by local destination) accumulates agg[d, w] += M.T @ S into PSUM per
W-node destination block; the 1/deg mean scaling is applied afterwards
as a per-destination-column multiply on the [D, W] block aggregate.
Source rows are fetched with dma_gather (int16 indices -> 4 source
chunks of 25000 rows). Linear layers and bias are applied per block on
TensorE. Layer 1 and layer 2 run as two launches; the host assembles
the full hidden table in between (the inter-core exchange).

Because S is a pure one-hot, SBATCH consecutive columns are built by a
single DVE tensor_tensor(is_equal) with stride-0 broadcast APs
(out[p,k,w] = (iota[w] == dst[p,k])), amortizing the ~300ns PSUM-access
init across the batch. One-hot codes live in alternating [0,W) / [W,2W)
windows by block parity, letting destination cells be 64-granular
instead of 128-aligned (gather padding 9% -> 4%): a 128-slot column
straddling two blocks is built once per block against that block's
window (foreign-parity rows match nothing) and every matmul runs full
128 rows. Partition-sliced matmuls (base 64) fault in this runtime, so
masking-by-parity is the only viable sub-128 scheme. The straddled-
column rebuilds (+~30us/layer DVE) fit inside DVE's slack under the
gather floor, so the padding reduction lands on the critical path.

Node->slot assignment: nodes are degree-sorted into bands of
n_cores*W; within each band a greedy 4-vector (per-source-chunk
in-degree) balancer splits nodes across the 8 cores so each
(block, chunk) cell has near-equal edge counts on every core. This
minimizes the shared gather padding (cell size = max over cores).

MODE "f16": W=192 destination windows; features, routing matrices and
weights in fp16 (PE: 1 cycle/row; PSUM accumulates fp32; ~5e-4 rel err).
Block outputs are produced transposed [out_d, W]; layer-1 output is
written fp16 and reused directly as the layer-2 gather table.
MODE "f32r": fp32r datapath (~1e-4). MODE "f32": everything fp32.

HW tuning (measured via differential loop_k timing on TRN2):
- dma_gather costs ~8.7us per 1024-idx instruction end-to-end on one
  SWDGE queue; issuing across all 4 queues (N_QUEUES) pipelines this
  3.8x (~2.3ns/row; the dominant cost at ~480us/layer). Gathers >1024
  idxs overflow the descriptor ring and wedge.
- The DVE's second SBUF read port is shared with GPSIMD; while GPSIMD
  generates gather descriptors, 2-port DVE ops stall. The batched
  is_equal reads iota via the PSUM port and dst via SBUF port 1, so
  port 2 stays free for descriptor-gen.
- relu/copy of block outputs runs on the ACT engine (ACT_FIN). With
  the batched DVE one-hots, offloading s-columns to ACT (2-op Abs/Relu
  form, ~2.3us/col) HURTS: stile-only measured 523us at
  STILE_ACT_FRAC=1/8 vs 407us at 0. All s-columns stay on DVE, which
  sits fully under the ~480us gather floor.

Measured totals (this test.py's differential loop_k method): baseline
2187/1910us -> batched one-hot 1137 -> act_frac=0 1029 -> 64-granular
cells 981us (L1 504 + L2 477; rel err 4.9e-04).
"""

import contextlib
import sys

sys.path.insert(0, "/opt/trn_rl_repo")

import numpy as np

import concourse.mybir as mybir
import concourse.tile as tile
from concourse import bacc, bass_utils

N_NODES = 100000
N_EDGES = 1600000
IN_DIM = 128
HID_DIM = 128
OUT_DIM = 64
N_CORES = 8
N_CHUNKS = 4
CHUNK_SZ = 25000
GATHER_MAX = 1024  # HW limit: dma_gather wedges above this
N_QUEUES = 4       # SWDGE queues (ucode max 4); 1 queue serializes the
                   # ~8.7us per-gather round trip, 4 queues pipeline it

MODE = "f16"       # "f32" | "f32r" | "f16"
IOTA_MODE = "psum"  # s_tile in0 source; psum forces the DVE op into
                    # 1-port mode so it stops contending with GPSIMD
                    # gather descriptor-gen on the shared second SBUF
                    # port (big measured win)
ACT_FIN = True      # relu/copy of block outputs on ACT instead of DVE
SBATCH = 8          # s_tiles per DVE instruction: the routing matrices are
                    # pure one-hot (cnt_inv folded into a per-column scale
                    # at finalize), so k tiles batch into one
                    # tensor_tensor(is_equal) via stride-0 broadcast APs,
                    # amortizing the ~250-380ns PSUM-read init k-fold
STILE_ACT_FRAC = 0.0    # fraction (8ths) of s_tile batches built on ACT
                        # via u=Abs(iota-dst); s=Relu(1-u). With batched
                        # DVE one-hots, ACT (~2.3us/col) is the straggler:
                        # measured stile-only 523us at 1/8 vs 407us at 0 —
                        # all-DVE sits fully under the ~500us gather floor

_plan_cache: dict = {}
_prog_cache: dict = {}


def _block_w(mode):
    # f16: W=192 beats 256 — the s_tile DVE op (the critical chain) costs
    # W*1.04ns + ~250ns PSUM-read init per 128-edge tile, so narrower
    # windows win despite slightly more gather padding (8.9% vs 6.9%)
    if mode == "f16":
        return 192
    return 128 if mode == "f32" else 256


def _transposed_out(mode):
    return mode in ("f32r", "f16")


def _assign_slots(deg4, n_nodes, n_cores, W):
    """Greedy per-band balance of per-chunk degree vectors across cores.

    Returns (slot_of_node, n_bands). Band j holds the degree-ranked nodes
    [j*n_cores*W, ...); within the band each core gets W nodes chosen to
    equalize the per-chunk edge counts (which become the gather cells).
    """
    tot = deg4.sum(1)
    order = np.argsort(-tot, kind="stable")
    band_sz = n_cores * W
    n_bands = -(-n_nodes // band_sz)
    spc = n_bands * W
    slot_of_node = np.empty(n_nodes, np.int64)
    BIG = np.int64(1) << 60
    for j in range(n_bands):
        nodes = order[j * band_sz: (j + 1) * band_sz]
        s = np.zeros((n_cores, deg4.shape[1]), np.int64)
        cap = np.zeros(n_cores, np.int64)
        for n_ in nodes:
            v = deg4[n_]
            cand = s + v
            post = np.maximum(cand, s.max(0))
            score = post.sum(1)
            score[cap >= W] = BIG
            k = int(score.argmin())
            slot_of_node[n_] = k * spc + j * W + cap[k]
            s[k] += v
            cap[k] += 1
    return slot_of_node, n_bands


def _make_plan(edge_index, n_nodes, n_cores, chunk_sz, n_chunks, mode=MODE,
               gather_max=GATHER_MAX):
    src = np.asarray(edge_index[0], dtype=np.int64)
    dst = np.asarray(edge_index[1], dtype=np.int64)
    n_edges = src.shape[0]
    W = _block_w(mode)

    deg = np.bincount(dst, minlength=n_nodes).astype(np.int64)
    cnt_inv = (1.0 / np.maximum(deg, 1)).astype(np.float32)

    chunk_e = src // chunk_sz
    deg4 = np.bincount(dst * n_chunks + chunk_e,
                       minlength=n_nodes * n_chunks
                       ).reshape(n_nodes, n_chunks)
    slot_of_node, bpc = _assign_slots(deg4, n_nodes, n_cores, W)
    slots_per_core = bpc * W

    dslot = slot_of_node[dst]
    core_e = dslot // slots_per_core
    blk_e = (dslot % slots_per_core) // W
    dloc_e = dslot % W

    cell = (core_e * bpc + blk_e) * n_chunks + chunk_e
    n_cells = n_cores * bpc * n_chunks
    counts = np.bincount(cell, minlength=n_cells).reshape(
        n_cores, bpc, n_chunks)

    # cells are 64-granular (SBUF AP partition bases may only be 0/32/64,
    # and 64-row matmul pieces need base 0 or 64); chunk segments are
    # 128-aligned so gather-buffer partitions line up with the global
    # slot%128 <-> dst_sb row mapping. Cells need NOT be 128-aligned:
    # matmuls run on partition-sliced pieces of each 128-slot column.
    # 16-granular cells: ~1.5% gather padding vs ~9% at 128-aligned, ~5%
    # at 64-granular. The straddled-column rebuilds (~7/8 of block
    # boundaries at this granularity) fit in DVE's slack now that the
    # whole s_tile path runs on DVE (~450us vs the ~465us gather floor),
    # so the gather savings land directly on the critical path. 16 is the
    # floor: gather idx slices require 16-multiple offsets/counts.
    cell_slots = (-(-counts.max(axis=0) // 16) * 16).astype(np.int64)
    seg_len = cell_slots.sum(axis=0)             # per chunk
    seg_len = (-(-seg_len // 128) * 128).astype(np.int64)
    seg_start = np.concatenate([[0], np.cumsum(seg_len)[:-1]])
    cell_base = np.empty((bpc, n_chunks), np.int64)
    for c in range(n_chunks):
        cell_base[:, c] = seg_start[c] + np.concatenate(
            [[0], np.cumsum(cell_slots[:, c])[:-1]])
    total_slots = int(seg_len.sum())

    gathers = []
    for c in range(n_chunks):
        lst = []
        off = 0
        while off < seg_len[c]:
            n = int(min(gather_max, seg_len[c] - off))
            lst.append((int(seg_start[c] + off), n))
            off += n
        gathers.append(lst)

    # slot position of every edge
    eorder = np.argsort(cell, kind="stable")
    sorted_cell = cell[eorder]
    group_start = np.zeros(n_edges, np.int64)
    new_grp = np.empty(n_edges, bool)
    new_grp[0] = True
    new_grp[1:] = sorted_cell[1:] != sorted_cell[:-1]
    grp_first = np.where(new_grp)[0]
    group_start[grp_first] = grp_first
    group_start = np.maximum.accumulate(group_start)
    rank = np.arange(n_edges) - group_start

    b_of = (sorted_cell // n_chunks) % bpc
    c_of = sorted_cell % n_chunks
    core_of = sorted_cell // (bpc * n_chunks)
    pos = cell_base[b_of, c_of] + rank

    idx_vals = np.zeros((n_cores, total_slots), np.int16)
    dloc_vals = np.full((n_cores, total_slots), -1.0, np.float32)
    cinv_vals = np.zeros((n_cores, total_slots), np.float32)

    es, ed = src[eorder], dst[eorder]
    idx_vals[core_of, pos] = (es - c_of * chunk_sz).astype(np.int16)
    # one-hot codes live in alternating windows [0,W) / [W,2W) by block
    # parity, so a 128-slot column straddling two blocks can be built
    # per-block with full-row matmuls (foreign-parity rows match nothing)
    dloc_vals[core_of, pos] = (dloc_e[eorder]
                               + (b_of % 2) * W).astype(np.float32)
    cinv_vals[core_of, pos] = cnt_inv[ed]
    # a column may straddle at most two cells; an empty cell between two
    # same-parity cells would break the parity masking
    assert counts.max(axis=0).min() > 0

    idx16 = np.ascontiguousarray(
        np.tile(idx_vals.reshape(n_cores, -1, 16).transpose(0, 2, 1),
                (1, 8, 1)))
    dstloc = np.ascontiguousarray(
        dloc_vals.reshape(n_cores, -1, 128).transpose(0, 2, 1))
    cntinv = np.ascontiguousarray(
        cinv_vals.reshape(n_cores, -1, 128).transpose(0, 2, 1))

    # per-destination-slot 1/deg, for the column scale at finalize
    cnt_slot = np.ones((n_cores * slots_per_core,), np.float32)
    cnt_slot[slot_of_node] = cnt_inv
    cnt_slot = cnt_slot.reshape(n_cores, slots_per_core)

    return dict(
        slot_of_node=slot_of_node, bpc=bpc, slots_per_core=slots_per_core,
        cell16=cell_slots, gathers=gathers, total_slots=total_slots,
        cell_base=cell_base, seg_start=seg_start, mode=mode, W=W,
        idx16=idx16, dstloc=dstloc, cntinv=cntinv, cnt_slot=cnt_slot,
        chunk_sz=chunk_sz,
        n_chunks=n_chunks, n_nodes=n_nodes, n_cores=n_cores,
        gather_max=gather_max,
    )


def _feat_np_dtype(mode):
    return np.float16 if mode == "f16" else np.float32


def _build_program(plan, table_rows, out_d, relu, loop_k=1, out_f32=None,
                   ablate=(), n_queues=N_QUEUES, gbufs=10, sbufs=6, pbufs=3,
                   iota_mode=None, act_fin=None, act_frac=None, sbatch=None):
    """One layer's SPMD program (shared by all cores).

    loop_k > 1 wraps the block loop in a hardware For loop repeating the
    computation loop_k times (timing only). out_f32 forces the DRAM
    output dtype (default: f32 unless mode f16 and relu, i.e. layer 1,
    whose output feeds the next layer's f16 gather table).
    ablate: timing-ablation set for microbenchmarks; any of
    {"gather", "stile", "matmul", "finalize"} drops that component
    (results become garbage; timing only).
    """
    ablate = set(ablate)
    bpc = plan["bpc"]
    cell16 = plan["cell16"]
    n_chunks = plan["n_chunks"]
    chunk_sz = plan["chunk_sz"]
    total_slots = plan["total_slots"]
    slots_pc = plan["slots_per_core"]
    gathers = plan["gathers"]
    cell_base = plan["cell_base"]
    seg_start = plan["seg_start"]
    mode = plan["mode"]
    W = plan["W"]
    D = 128
    f32 = mybir.dt.float32
    if mode == "f32r":
        mdt = mybir.dt.float32r
    elif mode == "f16":
        mdt = mybir.dt.float16
    else:
        mdt = f32
    if out_f32 is None:
        out_f32 = not (mode == "f16" and relu)
    out_dt = f32 if out_f32 else mdt
    # self-term inputs: f16 mode runs them in f16 (1 cyc/row); f32r keeps f32
    sdt = mdt if mode == "f16" else f32
    gather_max = plan.get("gather_max", GATHER_MAX)
    if iota_mode is None:
        iota_mode = IOTA_MODE
    assert iota_mode == "psum", "batched s_tile path reads the PSUM iota"
    if act_fin is None:
        act_fin = ACT_FIN
    if act_frac is None:
        act_frac = STILE_ACT_FRAC
    if sbatch is None:
        sbatch = SBATCH
    act_eighths = int(round(act_frac * 8))
    if act_eighths > 0 and pbufs > 2:
        pbufs = 2  # psU needs 2 PSUM banks; 8-bank budget
    iota_cols = 2 * W  # two parity windows (see _make_plan dloc comment)
    # per-slot 1/deg column-scale table; f16 halves its SBUF footprint
    # (25KB/partition) and 1/deg in f16 adds ~2e-4 relative error
    cdt = mybir.dt.float16

    nc = bacc.Bacc(
        "TRN2", target_bir_lowering=False, debug=False,
        dynamic_dma_scratch_size=max(16384, 16 * gather_max),
        num_swdge_queues=n_queues,
    )
    with tile.TileContext(nc) as tc:
        with tc.tile_pool(name="dram", bufs=1, space="DRAM") as dram:
            table = dram.tile([table_rows, D], mdt,
                              kind="ExternalInput", name="table")
            idx16 = dram.tile([128, total_slots // 16], mybir.dt.int16,
                              kind="ExternalInput", name="idx16")
            dstloc = dram.tile([128, total_slots // 128], f32,
                               kind="ExternalInput", name="dstloc")
            cntbT = dram.tile([128, slots_pc], cdt,
                              kind="ExternalInput", name="cntbT")
            xT = dram.tile([D, slots_pc], sdt,
                           kind="ExternalInput", name="xT")
            wl = dram.tile([D, out_d], mdt,
                           kind="ExternalInput", name="wl")
            wr = dram.tile([D, out_d], sdt,
                           kind="ExternalInput", name="wr")
            brow = dram.tile([1, out_d], mdt,
                             kind="ExternalInput", name="brow")
            iota_in = dram.tile([128, iota_cols], mdt,
                                kind="ExternalInput", name="iota")
            onesr = dram.tile([1, W], mdt,
                              kind="ExternalInput", name="onesr")
            if _transposed_out(mode):
                out = dram.tile([out_d, slots_pc], out_dt,
                                kind="ExternalOutput", name="out")
            else:
                out = dram.tile([slots_pc, out_d], out_dt,
                                kind="ExternalOutput", name="out")

        with tc.tile_pool(name="const", bufs=1) as cpool, \
             tc.tile_pool(name="gbuf", bufs=gbufs) as gpool, \
             tc.tile_pool(name="spool", bufs=sbufs) as spool, \
             tc.tile_pool(name="fpool", bufs=3) as fpool, \
             tc.tile_pool(name="psA", bufs=pbufs, space="PSUM") as psA, \
             tc.tile_pool(name="psB", bufs=pbufs, space="PSUM") as psB, \
             tc.tile_pool(name="psI", bufs=1, space="PSUM") as psI, \
             tc.tile_pool(name="psU", bufs=2, space="PSUM") as psU:

            idx_sb = cpool.tile([128, total_slots // 16], mybir.dt.int16)
            dst_sb = cpool.tile([128, total_slots // 128], f32)
            cntb_sb = cpool.tile([128, slots_pc], cdt)
            xT_sb = cpool.tile([D, slots_pc], sdt)
            wl_sb = cpool.tile([D, out_d], mdt)
            wr_sb = cpool.tile([D, out_d], sdt)
            b_sb = cpool.tile([1, out_d], mdt)
            ones_sb = cpool.tile([1, W], mdt)
            iota_sb = cpool.tile([128, iota_cols], mdt)

            nc.sync.dma_start(out=idx_sb[:], in_=idx16[:])
            nc.sync.dma_start(out=dst_sb[:], in_=dstloc[:])
            nc.sync.dma_start(out=cntb_sb[:], in_=cntbT[:])
            nc.sync.dma_start(out=xT_sb[:], in_=xT[:])
            nc.sync.dma_start(out=wl_sb[:], in_=wl[:])
            nc.sync.dma_start(out=wr_sb[:], in_=wr[:])
            nc.sync.dma_start(out=b_sb[:], in_=brow[:])
            nc.sync.dma_start(out=iota_sb[:], in_=iota_in[:])
            nc.sync.dma_start(out=ones_sb[:], in_=onesr[:])

            # s_tile in0 source: DVE reads the iota via its dedicated PSUM
            # port (1x mode); in1 (dst) uses SBUF read port 1. Port 2 —
            # shared with GPSIMD desc-gen — stays untouched.
            iota_ps = psI.tile([128, 2 * W], f32, space="PSUM",
                               tag="iotaP", name="iotaP")
            nc.scalar.copy(out=iota_ps[:], in_=iota_sb[:])

            loop_ctx = (tc.For_i(0, loop_k, 1) if loop_k > 1
                        else contextlib.nullcontext())

            gtiles = [dict() for _ in range(n_chunks)]
            next_g = [0] * n_chunks

            gcounter = [0]

            def ensure_gather(c, gi):
                while next_g[c] <= gi:
                    g = next_g[c]
                    s0, n = gathers[c][g]
                    gb = gpool.tile([128, gather_max // 128, D], mdt,
                                    tag=f"g{c}", name=f"gb_{c}_{g}")
                    if "gather" not in ablate:
                        nc.gpsimd.dma_gather(
                            out_ap=gb[:, : -(-n // 128), :],
                            in_ap=table[c * chunk_sz : min((c + 1) * chunk_sz,
                                                           table_rows), :],
                            idxs_ap=idx_sb[:, s0 // 16 : (s0 + n) // 16],
                            num_idxs=n,
                            num_idxs_reg=n,
                            elem_size=D,
                            queue_num=gcounter[0] % n_queues,
                        )
                        gcounter[0] += 1
                    gtiles[c][g] = gb
                    next_g[c] = g + 1

            stack = contextlib.ExitStack()
            stack.enter_context(loop_ctx)
            bcount = 0
            for b in range(bpc):
                if "matmul" not in ablate:
                    agg = psA.tile([D, W], f32, space="PSUM",
                                   tag="agg", name=f"agg_{b}")
                # this block's one-hot codes live in parity window
                # [off, off+W); foreign-parity rows of straddled columns
                # match nothing, so every matmul runs full 128 rows
                off = (b % 2) * W
                iota_src = iota_ps[:, off : off + W]
                # columns of the 128-slot grid this block's cells touch
                cols = []
                for c in range(n_chunks):
                    cnt = int(cell16[b, c])
                    if cnt == 0:
                        continue
                    base = int(cell_base[b, c])
                    end = base + cnt
                    for col in range(base // 128, (end - 1) // 128 + 1):
                        cols.append((c, col))
                n_mm = len(cols)

                # build s columns, batched over consecutive columns
                scol = {}
                runs = []
                for c, col in cols:
                    if (runs and col == runs[-1][-1] + 1
                            and len(runs[-1]) < sbatch):
                        runs[-1].append(col)
                    else:
                        runs.append([col])
                for run in runs:
                    nb = len(run)
                    col0 = run[0]
                    sb = spool.tile([128, nb * W], mdt,
                                    tag="s", name=f"s_{b}_{col0}")
                    use_act = (act_eighths > 0
                               and bcount % 8 < act_eighths)
                    bcount += 1
                    if "stile" not in ablate:
                        if use_act:
                            # ACT path (per column): u=|iota-dst|;
                            # s=relu(1-u) == 1 iff u==0
                            for i in range(nb):
                                u_ps = psU.tile(
                                    [128, W], f32, space="PSUM",
                                    tag="u", name=f"u_{b}_{col0 + i}")
                                nc.scalar.activation(
                                    out=u_ps[:], in_=iota_src,
                                    func=mybir.ActivationFunctionType.Abs,
                                    bias=dst_sb[:, col0 + i : col0 + i + 1],
                                    scale=-1.0)
                                nc.scalar.activation(
                                    out=sb[:, i * W : (i + 1) * W],
                                    in_=u_ps[:],
                                    func=mybir.ActivationFunctionType.Relu,
                                    bias=1.0, scale=-1.0)
                        else:
                            # one DVE op builds nb one-hot s columns:
                            # out[p,k,w] = (iota[off+w] == dst[p,k])
                            nc.vector.tensor_tensor(
                                out=sb[:].rearrange(
                                    "p (k w) -> p k w", k=nb),
                                in0=iota_src.unsqueeze(1).to_broadcast(
                                    [128, nb, W]),
                                in1=dst_sb[:, col0 : col0 + nb]
                                    .unsqueeze(2).to_broadcast(
                                        [128, nb, W]),
                                op=mybir.AluOpType.is_equal,
                            )
                    for i, col in enumerate(run):
                        scol[col] = (sb, i)

                mm = 0
                for c, col in cols:
                    slot0 = col * 128
                    g = (slot0 - int(seg_start[c])) // gather_max
                    tin = ((slot0 - int(seg_start[c]))
                           % gather_max) // 128
                    ensure_gather(c, g)
                    gb = gtiles[c][g]
                    if "matmul" not in ablate:
                        sbt, li = scol[col]
                        nc.tensor.matmul(
                            out=agg[:],
                            lhsT=gb[:, tin, :],
                            rhs=sbt[:, li * W : (li + 1) * W],
                            start=(mm == 0),
                            stop=(mm == n_mm - 1),
                        )
                    mm += 1

                if "finalize" in ablate:
                    fin = fpool.tile([out_d, W], out_dt,
                                     tag="fin", name=f"fin_{b}")
                    nc.scalar.copy(out=fin[:], in_=iota_sb[:out_d, :W])
                    nc.sync.dma_start(out=out[:, b * W : (b + 1) * W],
                                      in_=fin[:])
                elif _transposed_out(mode):
                    # transposed finalize: outp [out_d, W]
                    outp = psB.tile([out_d, W], f32, space="PSUM",
                                    tag="outp", name=f"outp_{b}")
                    if n_mm > 0:
                        aggc = fpool.tile([D, W], mdt,
                                          tag="aggc", name=f"aggc_{b}")
                        # mean: scale each dst column by its 1/deg here
                        # (the s_tiles are pure one-hot)
                        nc.vector.tensor_tensor(
                            out=aggc[:], in0=agg[:],
                            in1=cntb_sb[:, b * W : (b + 1) * W],
                            op=mybir.AluOpType.mult)
                        nc.tensor.matmul(out=outp[:], lhsT=wl_sb[:],
                                         rhs=aggc[:], start=True, stop=False)
                        nc.tensor.matmul(
                            out=outp[:], lhsT=wr_sb[:],
                            rhs=xT_sb[:, b * W : (b + 1) * W],
                            start=False, stop=False)
                    else:
                        nc.tensor.matmul(
                            out=outp[:], lhsT=wr_sb[:],
                            rhs=xT_sb[:, b * W : (b + 1) * W],
                            start=True, stop=False)
                    nc.tensor.matmul(out=outp[:], lhsT=b_sb[:],
                                     rhs=ones_sb[:], start=False, stop=True)

                    fin = fpool.tile([out_d, W], out_dt,
                                     tag="fin", name=f"fin_{b}")
                    if act_fin:
                        if relu:
                            nc.scalar.activation(
                                out=fin[:], in_=outp[:],
                                func=mybir.ActivationFunctionType.Relu)
                        else:
                            nc.scalar.copy(out=fin[:], in_=outp[:])
                    elif relu:
                        nc.vector.tensor_scalar(
                            out=fin[:], in0=outp[:], scalar1=0.0,
                            scalar2=None, op0=mybir.AluOpType.max)
                    else:
                        nc.vector.tensor_copy(out=fin[:], in_=outp[:])
                    nc.sync.dma_start(out=out[:, b * W : (b + 1) * W],
                                      in_=fin[:])
                else:
                    outp = psB.tile([W, out_d], f32, space="PSUM",
                                    tag="outp", name=f"outp_{b}")
                    if n_mm > 0:
                        aggc = fpool.tile([D, W], f32,
                                          tag="aggc", name=f"aggc_{b}")
                        nc.vector.tensor_tensor(
                            out=aggc[:], in0=agg[:],
                            in1=cntb_sb[:, b * W : (b + 1) * W],
                            op=mybir.AluOpType.mult)
                        nc.tensor.matmul(out=outp[:], lhsT=aggc[:],
                                         rhs=wl_sb[:], start=True, stop=False)
                        nc.tensor.matmul(
                            out=outp[:],
                            lhsT=xT_sb[:, b * W : (b + 1) * W],
                            rhs=wr_sb[:], start=False, stop=False)
                    else:
                        nc.tensor.matmul(
                            out=outp[:],
                            lhsT=xT_sb[:, b * W : (b + 1) * W],
                            rhs=wr_sb[:], start=True, stop=False)
                    nc.tensor.matmul(out=outp[:], lhsT=ones_sb[:],
                                     rhs=b_sb[:], start=False, stop=True)

                    fin = fpool.tile([W, out_d], out_dt,
                                     tag="fin", name=f"fin_{b}")
                    if relu:
                        nc.vector.tensor_scalar(
                            out=fin[:], in0=outp[:], scalar1=0.0,
                            scalar2=None, op0=mybir.AluOpType.max)
                    else:
                        nc.vector.tensor_copy(out=fin[:], in_=outp[:])
                    nc.sync.dma_start(out=out[b * W : (b + 1) * W, :],
                                      in_=fin[:])
            stack.close()

    nc.compile()
    names = dict(table=table.name, idx16=idx16.name, dstloc=dstloc.name,
                 cntbT=cntbT.name, xT=xT.name,
                 wl=wl.name, wr=wr.name, brow=brow.name, iota=iota_in.name,
                 onesr=onesr.name, out=out.name)
    return nc, names


def _layer_in_maps(names, plan, table_np, xT_np, wlT, wrT, b_vec, out_d,
                   iota_mode=None):
    W = plan["W"]
    fdt = _feat_np_dtype(plan["mode"])
    iota = np.broadcast_to(np.arange(2 * W, dtype=fdt), (128, 2 * W)).copy()
    in_maps = []
    for c in range(plan["n_cores"]):
        cntb = np.ascontiguousarray(np.broadcast_to(
            plan["cnt_slot"][c], (128, plan["slots_per_core"])),
            dtype=np.float16)
        in_maps.append({
            names["table"]: np.ascontiguousarray(table_np, dtype=fdt),
            names["idx16"]: plan["idx16"][c],
            names["dstloc"]: plan["dstloc"][c],
            names["cntbT"]: cntb,
            names["xT"]: np.ascontiguousarray(xT_np[c], dtype=fdt),
            names["wl"]: np.ascontiguousarray(wlT, dtype=fdt),
            names["wr"]: np.ascontiguousarray(wrT, dtype=fdt),
            names["brow"]: np.ascontiguousarray(
                b_vec.reshape(1, out_d), dtype=fdt),
            names["iota"]: iota,
            names["onesr"]: np.ones((1, W), fdt),
        })
    return in_maps


def _run_layer(nc, names, plan, table_np, xT_np, wlT, wrT, b_vec, out_d):
    in_maps = _layer_in_maps(names, plan, table_np, xT_np, wlT, wrT, b_vec,
                             out_d)
    res = bass_utils.run_bass_kernel_spmd(
        nc, in_maps, core_ids=list(range(plan["n_cores"])))
    return [res.results[c][names["out"]] for c in range(plan["n_cores"])]


def _get_plan_and_progs(edge_index):
    key = (hash(edge_index.tobytes()), MODE)
    if key not in _plan_cache:
        _plan_cache[key] = _make_plan(edge_index, N_NODES, N_CORES,
                                      CHUNK_SZ, N_CHUNKS, MODE)
    plan = _plan_cache[key]
    if (key, "L1") not in _prog_cache:
        _prog_cache[(key, "L1")] = _build_program(plan, N_NODES, HID_DIM,
                                                  relu=True)
    if (key, "L2") not in _prog_cache:
        _prog_cache[(key, "L2")] = _build_program(plan, N_NODES, OUT_DIM,
                                                  relu=False)
    return plan, _prog_cache[(key, "L1")], _prog_cache[(key, "L2")]


def kernel(x, edge_index, W1l, b1, W1r, W2l, b2, W2r):
    x = np.asarray(x, np.float32)
    edge_index = np.asarray(edge_index)
    W1l = np.asarray(W1l, np.float32)
    b1 = np.asarray(b1, np.float32)
    W1r = np.asarray(W1r, np.float32)
    W2l = np.asarray(W2l, np.float32)
    b2 = np.asarray(b2, np.float32)
    W2r = np.asarray(W2r, np.float32)

    plan, (nc1, names1), (nc2, names2) = _get_plan_and_progs(edge_index)

    slot_of_node = plan["slot_of_node"]
    spc = plan["slots_per_core"]
    n_cores = plan["n_cores"]
    fdt = _feat_np_dtype(plan["mode"])

    xq = np.zeros((n_cores * spc, IN_DIM), fdt)
    xq[slot_of_node] = x.astype(fdt)
    xT_np = [np.ascontiguousarray(xq[c * spc : (c + 1) * spc].T)
             for c in range(n_cores)]

    h_parts = _run_layer(nc1, names1, plan, x, xT_np,
                         np.ascontiguousarray(W1l.T),
                         np.ascontiguousarray(W1r.T), b1, HID_DIM)

    if _transposed_out(plan["mode"]):
        # parts are transposed [hid, spc]
        hq = np.concatenate(h_parts, axis=1)          # [hid, n_cores*spc]
        h_full = np.ascontiguousarray(hq.T[slot_of_node])
        hT_np = [np.ascontiguousarray(p) for p in h_parts]
    else:
        hq = np.concatenate(h_parts, axis=0)
        h_full = np.ascontiguousarray(hq[slot_of_node])
        hT_np = [np.ascontiguousarray(h_parts[c].T) for c in range(n_cores)]

    out_parts = _run_layer(nc2, names2, plan, h_full, hT_np,
                           np.ascontiguousarray(W2l.T),
                           np.ascontiguousarray(W2r.T), b2, OUT_DIM)
    if _transposed_out(plan["mode"]):
        oq = np.concatenate(out_parts, axis=1)        # [out_d, total]
        return np.ascontiguousarray(oq.T[slot_of_node]).astype(np.float32)
    oq = np.concatenate(out_parts, axis=0)
    return np.ascontiguousarray(oq[slot_of_node]).astype(np.float32)

